# revision 1
# baseline (speedup 1.0000x reference)
"""TRN2 Bass/Tile kernel for nn_MultiHeadAttention_16647293239562.

kernel(x, Wqkv, Wo) -> [2, 2048, 1024] float32

Sharding: batch x head-group over 8 NeuronCores (core c: batch c//4, heads
4*(c%4)..4*(c%4)+3). Each core runs QKV projections (bf16 inputs, f32 PSUM
accumulation), RoPE (host-baked cos + pre-shifted sign-baked sin tables;
rotate-half via partition-shifted SBUF copies), full non-causal attention with
softmax denominators obtained from an extra ones-column in V (PV matmul row 64
= sum of exp), and a partial O-projection. Host sums 4 partials per batch.
"""

import sys

sys.path.insert(0, "/opt/trn_rl_repo")

import concourse.bass as bass
import concourse.mybir as mybir
import concourse.tile as tile
from concourse import bacc

F32 = mybir.dt.float32
F32R = mybir.dt.float32r
BF16 = mybir.dt.bfloat16
ts = bass.ts

L = 2048
D = 1024
DH = 64
NKT = D // 128   # 8 k-tiles
NLT = L // 128   # 16 l'-tiles
NQC = L // 512   # 4 q-chunks of 512
EXP = mybir.ActivationFunctionType.Exp


def build():
    nc = bacc.Bacc("TRN2", target_bir_lowering=False, debug=False)
    xTb = nc.dram_tensor("xTb", [D, L], BF16, kind="ExternalInput")
    wqkb = nc.dram_tensor("wqkb", [D, 512], BF16, kind="ExternalInput")
    wvb = nc.dram_tensor("wvb", [D, 256], BF16, kind="ExternalInput")
    wo = nc.dram_tensor("wo", [256, D], BF16, kind="ExternalInput")
    cosT = nc.dram_tensor("cosT", [128, L], F32, kind="ExternalInput")
    sinS = nc.dram_tensor("sinS", [128, L], F32, kind="ExternalInput")
    vones = nc.dram_tensor("vones", [128, NLT * 260], BF16, kind="ExternalInput")
    out_p = nc.dram_tensor("out_p", [L, D], F32, kind="ExternalOutput")

    with tile.TileContext(nc) as tc:
        with tc.tile_pool(name="persist", bufs=1) as pers, \
             tc.tile_pool(name="mmpool", bufs=2, space="PSUM") as mmp, \
             tc.tile_pool(name="spool", bufs=2, space="PSUM") as spool, \
             tc.tile_pool(name="opool", bufs=1, space="PSUM") as opool, \
             tc.tile_pool(name="projsb", bufs=3) as psb, \
             tc.tile_pool(name="attn", bufs=18) as ap, \
             tc.tile_pool(name="nrm", bufs=3) as nrm, \
             tc.tile_pool(name="ost", bufs=4) as ost:
            # ---- persistent SBUF ----
            qkT = pers.tile([128, 2, 2, L], BF16, name="qkT")  # [128, q/k, ptile, L]
            vx = pers.tile([128, NLT, 260], BF16, name="vx")
            vx4 = vx.rearrange("p t (h c) -> p t h c", c=65)
            outT = pers.tile([128, 2, L], BF16, name="outT")
            cos_sb = pers.tile([128, L], F32, name="cos_sb")
            sin_sb = pers.tile([128, L], F32, name="sin_sb")
            xTb_sb = pers.tile([128, NKT, L], BF16, name="xTb_sb")
            wqkb_sb = pers.tile([128, NKT, 512], BF16, name="wqkb_sb")
            wvb_sb = pers.tile([128, NKT, 256], BF16, name="wvb_sb")
            wo_sb = pers.tile([128, 2, D], BF16, name="wo_sb")

            # ---- loads: xTb/wqkb on the 2 HW queues (n-major), rest on SWDGE ----
            xTbr = xTb.rearrange("(k p) l -> p k l", p=128)
            wqkbr = wqkb.rearrange("(k p) m -> p k m", p=128)
            for kt in range(NKT):
                eng = nc.scalar if (kt % 2 == 0) else nc.gpsimd
                eng.dma_start(out=wqkb_sb[:, kt, :], in_=wqkbr[:, kt, :])
            nc.gpsimd.dma_start(out=cos_sb, in_=cosT[:, :])
            nc.gpsimd.dma_start(out=sin_sb, in_=sinS[:, :])
            for n in range(NQC):
                for kt in range(NKT):
                    eng = nc.scalar if (kt % 2 == 0) else nc.gpsimd
                    eng.dma_start(out=xTb_sb[:, kt, ts(n, 512)], in_=xTbr[:, kt, ts(n, 512)])
            nc.gpsimd.dma_start(out=wvb_sb, in_=wvb.rearrange("(k p) m -> p k m", p=128))
            nc.gpsimd.dma_start(out=vx, in_=vones.rearrange("p (t c) -> p t c", c=260))
            nc.gpsimd.dma_start(out=wo_sb, in_=wo.rearrange("(j p) d -> p j d", p=128))

            def qk_proj(m):
                # m: 0=q ptile0, 1=q ptile1, 2=k ptile0, 3=k ptile1
                qk, pt = (0, m) if m < 2 else (1, m - 2)
                for n in range(NQC):
                    ps = mmp.tile([128, 512], F32, tag="small", name="ps")
                    for kt in range(NKT):
                        nc.tensor.matmul(ps, wqkb_sb[:, kt, ts(m, 128)],
                                         xTb_sb[:, kt, ts(n, 512)],
                                         start=(kt == 0), stop=(kt == NKT - 1))
                    # u = ps * shift(sin); then shift(u) = shift(ps) * sin,
                    # so no separate PSUM->SBUF copy is needed for the shift
                    u = psb.tile([128, 512], F32, tag="u", name="u")
                    nc.vector.tensor_mul(u, ps, sin_sb[:, ts(n, 512)])
                    rh = psb.tile([128, 512], F32, tag="rh", name="rh")
                    for blk in range(4):
                        src_p = blk * 32 + (32 if blk % 2 == 0 else -32)
                        nc.sync.dma_start(out=rh[blk * 32:(blk + 1) * 32, :],
                                          in_=u[src_p:src_p + 32, :])
                    t0 = psb.tile([128, 512], F32, tag="t0", name="t0")
                    nc.vector.tensor_mul(t0, ps, cos_sb[:, ts(n, 512)])
                    nc.vector.tensor_add(qkT[:, qk, pt, ts(n, 512)], t0, rh)

            def v_proj(lt):
                pv = mmp.tile([128, 256], F32, tag="small", padded_shape=[128, 512], name="pv")
                for kt in range(NKT):
                    nc.tensor.matmul(pv, xTb_sb[:, kt, ts(lt, 128)],
                                     wvb_sb[:, kt, :],
                                     start=(kt == 0), stop=(kt == NKT - 1))
                nc.vector.tensor_copy(vx4[:, lt, :, 0:64],
                                      pv.rearrange("p (h c) -> p h c", c=64))

            # ptile0 q/k first so attention pt=0 can start early
            qk_proj(0)
            qk_proj(2)
            for lt in range(NLT):
                v_proj(lt)
            qk_proj(1)
            qk_proj(3)

            def emit_outproj(qc2):
                # ---- output projection for one q-range (4 l-tiles) ----
                for sub in range(4):
                    lt2 = qc2 * 4 + sub
                    for n in range(2):
                        po = mmp.tile([128, 512], F32, tag="small", name="po")
                        for j in range(2):
                            nc.tensor.matmul(po, outT[:, j, ts(lt2, 128)],
                                             wo_sb[:, j, ts(n, 512)],
                                             start=(j == 0), stop=(j == 1))
                        o_sb = ost.tile([128, 512], F32, tag="o", name="o_sb")
                        nc.vector.tensor_copy(o_sb, po)
                        seng = nc.sync if (lt2 + n) % 2 == 0 else nc.scalar
                        seng.dma_start(out=out_p[ts(lt2, 128), ts(n, 512)], in_=o_sb)

            # ---- attention (pt-outer), high priority so the scheduler runs
            # scores/exp ASAP and packs V/ptile1-proj into the exp gaps
            for pt in range(2):
                for qc in range(NQC):
                  with tc.high_priority():
                    aABs = []
                    oA = opool.tile([65, 512], F32, tag="oA", name="oA")
                    oB = opool.tile([65, 512], F32, tag="oB", name="oB")
                    # phase 1: scores (both heads into one 2-bank tile) + one exp
                    for lt in range(NLT):
                        sAB = spool.tile([128, 1024], F32, tag="sAB", name="sAB")
                        nc.tensor.matmul(sAB[:, 0:512],
                                         qkT[0:64, 1, pt, ts(lt, 128)],
                                         qkT[0:64, 0, pt, ts(qc, 512)],
                                         start=True, stop=True)
                        nc.tensor.matmul(sAB[:, 512:1024],
                                         qkT[64:128, 1, pt, ts(lt, 128)],
                                         qkT[64:128, 0, pt, ts(qc, 512)],
                                         start=True, stop=True)
                        aAB = ap.tile([128, 1024], BF16, tag="aAB", name="aAB")
                        nc.scalar.activation(aAB, sAB, EXP, scale=0.125)
                        aABs.append(aAB)
                    # phase 2: PV accumulation (ones-column gives denominators)
                    h0, h1 = 2 * pt, 2 * pt + 1
                    for lt in range(NLT):
                        nc.tensor.matmul(oA, vx4[:, lt, h0, 0:65], aABs[lt][:, 0:512],
                                         start=(lt == 0), stop=(lt == NLT - 1))
                        nc.tensor.matmul(oB, vx4[:, lt, h1, 0:65], aABs[lt][:, 512:1024],
                                         start=(lt == 0), stop=(lt == NLT - 1))
                    # copy accumulators to SBUF so the PSUM slots free fast
                    oc = nrm.tile([65, 512], F32, tag="oc", name="oc")
                    od = nrm.tile([65, 512], F32, tag="od", name="od")
                    nc.vector.tensor_copy(oc, oA)
                    nc.vector.tensor_copy(od, oB)
                    # normalize: out^T rows = o[0:64] * (1/o[64])
                    rA = nrm.tile([1, 512], F32, tag="rA", name="rA")
                    rB = nrm.tile([1, 512], F32, tag="rB", name="rB")
                    nc.vector.reciprocal(rA, oc[64:65, :])
                    nc.vector.reciprocal(rB, od[64:65, :])
                    rbA = nrm.tile([64, 512], F32, tag="rbA", name="rbA")
                    rbB = nrm.tile([64, 512], F32, tag="rbB", name="rbB")
                    nc.gpsimd.partition_broadcast(rbA, rA)
                    nc.gpsimd.partition_broadcast(rbB, rB)
                    nc.vector.tensor_mul(outT[0:64, pt, ts(qc, 512)], oc[0:64, :], rbA)
                    nB = nrm.tile([64, 512], BF16, tag="nB", name="nB")
                    nc.vector.tensor_mul(nB, od[0:64, :], rbB)
                    nc.sync.dma_start(out=outT[64:128, pt, ts(qc, 512)], in_=nB)

                    if pt == 1 and qc > 0:
                        emit_outproj(qc - 1)
            emit_outproj(NQC - 1)

    nc.finalize()
    return nc


def host_prepare(x, Wqkv, Wo):
    """Build the 8 per-core input maps from full inputs."""
    import numpy as np
    import ml_dtypes
    B = x.shape[0]
    inv_freq = 1.0 / (10000.0 ** (np.arange(0, DH, 2, dtype=np.float64) / DH))
    t = np.arange(L, dtype=np.float64)
    freqs = np.outer(t, inv_freq)               # [L, 32]
    emb = np.concatenate([freqs, freqs], -1)    # [L, 64]
    cosT_ = np.cos(emb).T                       # [64, L]
    sinT = np.sin(emb).T
    # pre-shifted sign-baked sin: g[dh] = (-sin if dh>=32 else +sin)[dh xor 32]
    sinSg = np.concatenate([sinT[32:64], -sinT[0:32]], axis=0)  # [64, L]
    cosT2 = np.ascontiguousarray(np.tile(cosT_, (2, 1)), dtype=np.float32)  # [128, L]
    sinS2 = np.ascontiguousarray(np.tile(sinSg, (2, 1)), dtype=np.float32)  # [128, L]
    vones = np.ones((128, NLT * 260), dtype=ml_dtypes.bfloat16)

    xTb_h = [np.ascontiguousarray(x[b].T).astype(ml_dtypes.bfloat16) for b in range(B)]
    in_maps = []
    for c in range(8):
        b, hg = divmod(c, 4)
        hs = slice(4 * hg * DH, 4 * (hg + 1) * DH)  # 4 heads' col range (256 wide)
        wq = Wqkv[:, 0 * D:1 * D][:, hs]
        wk = Wqkv[:, 1 * D:2 * D][:, hs]
        wv_ = Wqkv[:, 2 * D:3 * D][:, hs]
        in_maps.append({
            "xTb": xTb_h[b],
            "wqkb": np.ascontiguousarray(
                np.concatenate([wq, wk], axis=1)).astype(ml_dtypes.bfloat16),
            "wvb": np.ascontiguousarray(wv_).astype(ml_dtypes.bfloat16),
            "wo": np.ascontiguousarray(Wo[hs, :]).astype(ml_dtypes.bfloat16),
            "cosT": cosT2,
            "sinS": sinS2,
            "vones": vones,
        })
    return in_maps


def host_gather(results):
    """Sum partial outputs per batch -> [2, L, D] float32."""
    import numpy as np
    out = np.zeros((2, L, D), dtype=np.float32)
    for c, r in enumerate(results):
        out[c // 4] += r["out_p"]
    return out


_CACHED = {}


def kernel(x, Wqkv, Wo):
    import numpy as np
    from concourse.bass_utils import run_bass_kernel_spmd

    if "nc" not in _CACHED:
        _CACHED["nc"] = build()
    nc = _CACHED["nc"]
    in_maps = host_prepare(np.asarray(x), np.asarray(Wqkv), np.asarray(Wo))
    res = run_bass_kernel_spmd(nc, in_maps, core_ids=list(range(8)))
    return host_gather(res.results)



# revision 9
# speedup vs baseline: 1.1030x; 1.1030x over previous
"""TRN2 Bass/Tile kernel for nn_MultiHeadAttention_16647293239562.

kernel(x, Wqkv, Wo) -> [2, 2048, 1024] float32

Sharding: batch x head-group over 8 NeuronCores (core c: batch c//4, heads
4*(c%4)..4*(c%4)+3). Each core runs QKV projections (bf16 inputs, f32 PSUM
accumulation), RoPE (host-baked cos + pre-shifted sign-baked sin tables;
rotate-half via partition-shifted SBUF copies), full non-causal attention with
softmax denominators obtained from an extra ones-column in V (PV matmul row 64
= sum of exp), and a partial O-projection. Host sums 4 partials per batch.

Perf notes vs the original baseline:
- no DMA triggers on the scalar engine (it is saturated by exp; HWDGE
  triggers cost it ~670ns each) -> loads on sync + gpsimd, stores on gpsimd
- projections emitted n-chunk-outer so the in-order PE stream never blocks
  on an x chunk that is still loading while later-arriving work is ready
- softmax 1/denom: denominators DMA'd [1,1024]->[128,8] so one reciprocal
  runs with all DVE lanes active (a [1,512] reciprocal costs 3.3us)
- O-projection emitted with a lag of TWO chunks so its dependency on the
  serial normalize chain never stalls the in-order PE queue
- rotate-half shuffle DMAs in bf16 on the sync queue (half the bytes)
- V ones-column via on-device memset (saves a 1.3MB DMA); out_p in bf16
"""

import sys

sys.path.insert(0, "/opt/trn_rl_repo")

import concourse.bass as bass
import concourse.mybir as mybir
import concourse.tile as tile
from concourse import bacc

F32 = mybir.dt.float32
BF16 = mybir.dt.bfloat16
ts = bass.ts

L = 2048
D = 1024
DH = 64
NKT = D // 128   # 8 k-tiles
NLT = L // 128   # 16 l'-tiles
NQC = L // 512   # 4 q-chunks of 512
EXP = mybir.ActivationFunctionType.Exp


def build():
    nc = bacc.Bacc("TRN2", target_bir_lowering=False, debug=False)
    xTb = nc.dram_tensor("xTb", [D, L], BF16, kind="ExternalInput")
    wqkb = nc.dram_tensor("wqkb", [D, 512], BF16, kind="ExternalInput")
    wvb = nc.dram_tensor("wvb", [D, 256], BF16, kind="ExternalInput")
    wo = nc.dram_tensor("wo", [256, D], BF16, kind="ExternalInput")
    cosT = nc.dram_tensor("cosT", [128, L], F32, kind="ExternalInput")
    sinS = nc.dram_tensor("sinS", [128, L], F32, kind="ExternalInput")
    out_p = nc.dram_tensor("out_p", [L, D], BF16, kind="ExternalOutput")

    with tile.TileContext(nc) as tc:
        with tc.tile_pool(name="persist", bufs=1) as pers, \
             tc.tile_pool(name="mmpool", bufs=2, space="PSUM") as mmp, \
             tc.tile_pool(name="spool", bufs=2, space="PSUM") as spool, \
             tc.tile_pool(name="opool", bufs=1, space="PSUM") as opool, \
             tc.tile_pool(name="projsb", bufs=3) as psb, \
             tc.tile_pool(name="attn", bufs=18) as ap, \
             tc.tile_pool(name="nrm", bufs=3) as nrm, \
             tc.tile_pool(name="ost", bufs=4) as ost:
            # ---- persistent SBUF ----
            qkT = pers.tile([128, 2, 2, L], BF16, name="qkT")  # [128, q/k, ptile, L]
            vx = pers.tile([128, NLT, 260], BF16, name="vx")
            vx4 = vx.rearrange("p t (h c) -> p t h c", c=65)
            outT = pers.tile([128, 2, L], BF16, name="outT")
            cos_sb = pers.tile([128, L], F32, name="cos_sb")
            sin_sb = pers.tile([128, L], F32, name="sin_sb")
            xTb_sb = pers.tile([128, NKT, L], BF16, name="xTb_sb")
            wqkb_sb = pers.tile([128, NKT, 512], BF16, name="wqkb_sb")
            wvb_sb = pers.tile([128, NKT, 256], BF16, name="wvb_sb")
            wo_sb = pers.tile([128, 2, D], BF16, name="wo_sb")

            # ---- loads on sync HWDGE + gpsimd SWDGE queues (never scalar) ----
            xTbr = xTb.rearrange("(k p) l -> p k l", p=128)
            wqkbr = wqkb.rearrange("(k p) m -> p k m", p=128)
            for kt in range(NKT):
                eng = nc.sync if (kt % 2 == 0) else nc.gpsimd
                eng.dma_start(out=wqkb_sb[:, kt, :], in_=wqkbr[:, kt, :])
            nc.gpsimd.dma_start(out=cos_sb, in_=cosT[:, :])
            nc.gpsimd.dma_start(out=sin_sb, in_=sinS[:, :])
            for n in range(NQC):
                for kt in range(NKT):
                    eng = nc.sync if (kt % 2 == 0) else nc.gpsimd
                    eng.dma_start(out=xTb_sb[:, kt, ts(n, 512)], in_=xTbr[:, kt, ts(n, 512)])
            nc.gpsimd.dma_start(out=wvb_sb, in_=wvb.rearrange("(k p) m -> p k m", p=128))
            nc.gpsimd.memset(vx4[:, :, :, 64:65], 1.0)  # ones column for denominators
            nc.gpsimd.dma_start(out=wo_sb, in_=wo.rearrange("(j p) d -> p j d", p=128))

            def qk_proj(m, n):
                # m: 0=q ptile0, 1=q ptile1, 2=k ptile0, 3=k ptile1
                qk, pt = (0, m) if m < 2 else (1, m - 2)
                ps = mmp.tile([128, 512], F32, tag="small", name="ps")
                for kt in range(NKT):
                    nc.tensor.matmul(ps, wqkb_sb[:, kt, ts(m, 128)],
                                     xTb_sb[:, kt, ts(n, 512)],
                                     start=(kt == 0), stop=(kt == NKT - 1))
                # u = ps * shift(sin); then shift(u) = shift(ps) * sin,
                # so no separate PSUM->SBUF copy is needed for the shift
                u = psb.tile([128, 512], BF16, tag="u", name="u")
                nc.vector.tensor_mul(u, ps, sin_sb[:, ts(n, 512)])
                rh = psb.tile([128, 512], BF16, tag="rh", name="rh")
                for blk in range(4):
                    src_p = blk * 32 + (32 if blk % 2 == 0 else -32)
                    nc.sync.dma_start(out=rh[blk * 32:(blk + 1) * 32, :],
                                      in_=u[src_p:src_p + 32, :])
                t0 = psb.tile([128, 512], BF16, tag="t0", name="t0")
                nc.vector.tensor_mul(t0, ps, cos_sb[:, ts(n, 512)])
                nc.vector.tensor_add(qkT[:, qk, pt, ts(n, 512)], t0, rh)

            def v_proj(lt):
                pv = mmp.tile([128, 256], F32, tag="small", padded_shape=[128, 512], name="pv")
                for kt in range(NKT):
                    nc.tensor.matmul(pv, xTb_sb[:, kt, ts(lt, 128)],
                                     wvb_sb[:, kt, :],
                                     start=(kt == 0), stop=(kt == NKT - 1))
                nc.vector.tensor_copy(vx4[:, lt, :, 0:64],
                                      pv.rearrange("p (h c) -> p h c", c=64))

            # n-chunk-outer emission: everything that needs x chunk n is
            # grouped, so early chunks' work never queues behind later loads.
            # ptile0 q/k first within each n so attention pt=0 starts early.
            for n in range(NQC):
                qk_proj(0, n)
                qk_proj(2, n)
                for lt in range(4 * n, 4 * n + 4):
                    v_proj(lt)
                qk_proj(1, n)
                qk_proj(3, n)

            def emit_outproj(qc2):
                # ---- output projection for one q-range (4 l-tiles) ----
                for sub in range(4):
                    lt2 = qc2 * 4 + sub
                    for n in range(2):
                        po = mmp.tile([128, 512], F32, tag="small", name="po")
                        for j in range(2):
                            nc.tensor.matmul(po, outT[:, j, ts(lt2, 128)],
                                             wo_sb[:, j, ts(n, 512)],
                                             start=(j == 0), stop=(j == 1))
                        o_sb = ost.tile([128, 512], BF16, tag="o", name="o_sb")
                        nc.vector.tensor_copy(o_sb, po)
                        nc.gpsimd.dma_start(out=out_p[ts(lt2, 128), ts(n, 512)], in_=o_sb)

            # ---- attention (pt-outer), high priority so the scheduler runs
            # scores/exp ASAP and packs V/ptile1-proj into the exp gaps
            for pt in range(2):
                for qc in range(NQC):
                  with tc.high_priority():
                    aABs = []
                    oA = opool.tile([65, 512], F32, tag="oA", name="oA")
                    oB = opool.tile([65, 512], F32, tag="oB", name="oB")
                    # phase 1: scores (both heads into one 2-bank tile) + one exp
                    for lt in range(NLT):
                        sAB = spool.tile([128, 1024], F32, tag="sAB", name="sAB")
                        nc.tensor.matmul(sAB[:, 0:512],
                                         qkT[0:64, 1, pt, ts(lt, 128)],
                                         qkT[0:64, 0, pt, ts(qc, 512)],
                                         start=True, stop=True)
                        nc.tensor.matmul(sAB[:, 512:1024],
                                         qkT[64:128, 1, pt, ts(lt, 128)],
                                         qkT[64:128, 0, pt, ts(qc, 512)],
                                         start=True, stop=True)
                        aAB = ap.tile([128, 1024], BF16, tag="aAB", name="aAB")
                        nc.scalar.activation(aAB, sAB, EXP, scale=0.125)
                        aABs.append(aAB)
                    # phase 2: PV accumulation (ones-column gives denominators)
                    h0, h1 = 2 * pt, 2 * pt + 1
                    for lt in range(NLT):
                        nc.tensor.matmul(oA, vx4[:, lt, h0, 0:65], aABs[lt][:, 0:512],
                                         start=(lt == 0), stop=(lt == NLT - 1))
                        nc.tensor.matmul(oB, vx4[:, lt, h1, 0:65], aABs[lt][:, 512:1024],
                                         start=(lt == 0), stop=(lt == NLT - 1))
                    # copy accumulators to SBUF so the PSUM slots free fast
                    ocd = nrm.tile([65, 1024], F32, tag="ocd", name="ocd")
                    nc.vector.tensor_copy(ocd[:, 0:512], oA)
                    nc.vector.tensor_copy(ocd[:, 512:1024], oB)
                    # normalize: out^T rows = o[0:64] * (1/o[64]); denominators
                    # go via [128,8] so the reciprocal uses all DVE lanes
                    dT = nrm.tile([128, 8], F32, tag="dT", name="dT")
                    nc.sync.dma_start(out=dT, in_=ocd[64:65, :])
                    rT = nrm.tile([128, 8], F32, tag="rT", name="rT")
                    nc.vector.reciprocal(rT, dT)
                    rAB = nrm.tile([1, 1024], F32, tag="rAB", name="rAB")
                    nc.sync.dma_start(out=rAB, in_=rT)
                    rbA = nrm.tile([64, 512], F32, tag="rbA", name="rbA")
                    rbB = nrm.tile([64, 512], F32, tag="rbB", name="rbB")
                    nc.gpsimd.partition_broadcast(rbA, rAB[:, 0:512])
                    nc.gpsimd.partition_broadcast(rbB, rAB[:, 512:1024])
                    nc.vector.tensor_mul(outT[0:64, pt, ts(qc, 512)], ocd[0:64, 0:512], rbA)
                    nB = nrm.tile([64, 512], BF16, tag="nB", name="nB")
                    nc.vector.tensor_mul(nB, ocd[0:64, 512:1024], rbB)
                    nc.sync.dma_start(out=outT[64:128, pt, ts(qc, 512)], in_=nB)

                    # lag-2 O-projection: its normalize deps resolved a full
                    # chunk ago, so it never stalls the in-order PE queue
                    if pt == 1 and qc >= 2:
                        emit_outproj(qc - 2)
            emit_outproj(NQC - 2)
            emit_outproj(NQC - 1)

    nc.finalize()
    return nc


def host_prepare(x, Wqkv, Wo):
    """Build the 8 per-core input maps from full inputs."""
    import numpy as np
    import ml_dtypes
    B = x.shape[0]
    inv_freq = 1.0 / (10000.0 ** (np.arange(0, DH, 2, dtype=np.float64) / DH))
    t = np.arange(L, dtype=np.float64)
    freqs = np.outer(t, inv_freq)               # [L, 32]
    emb = np.concatenate([freqs, freqs], -1)    # [L, 64]
    cosT_ = np.cos(emb).T                       # [64, L]
    sinT = np.sin(emb).T
    # pre-shifted sign-baked sin: g[dh] = (-sin if dh>=32 else +sin)[dh xor 32]
    sinSg = np.concatenate([sinT[32:64], -sinT[0:32]], axis=0)  # [64, L]
    cosT2 = np.ascontiguousarray(np.tile(cosT_, (2, 1)), dtype=np.float32)  # [128, L]
    sinS2 = np.ascontiguousarray(np.tile(sinSg, (2, 1)), dtype=np.float32)  # [128, L]

    xTb_h = [np.ascontiguousarray(x[b].T).astype(ml_dtypes.bfloat16) for b in range(B)]
    in_maps = []
    for c in range(8):
        b, hg = divmod(c, 4)
        hs = slice(4 * hg * DH, 4 * (hg + 1) * DH)  # 4 heads' col range (256 wide)
        wq = Wqkv[:, 0 * D:1 * D][:, hs]
        wk = Wqkv[:, 1 * D:2 * D][:, hs]
        wv_ = Wqkv[:, 2 * D:3 * D][:, hs]
        in_maps.append({
            "xTb": xTb_h[b],
            "wqkb": np.ascontiguousarray(
                np.concatenate([wq, wk], axis=1)).astype(ml_dtypes.bfloat16),
            "wvb": np.ascontiguousarray(wv_).astype(ml_dtypes.bfloat16),
            "wo": np.ascontiguousarray(Wo[hs, :]).astype(ml_dtypes.bfloat16),
            "cosT": cosT2,
            "sinS": sinS2,
        })
    return in_maps


def host_gather(results):
    """Sum partial outputs per batch -> [2, L, D] float32."""
    import numpy as np
    out = np.zeros((2, L, D), dtype=np.float32)
    for c, r in enumerate(results):
        out[c // 4] += r["out_p"].astype(np.float32)
    return out


_CACHED = {}


def kernel(x, Wqkv, Wo):
    import numpy as np
    from concourse.bass_utils import run_bass_kernel_spmd

    if "nc" not in _CACHED:
        _CACHED["nc"] = build()
    nc = _CACHED["nc"]
    in_maps = host_prepare(np.asarray(x), np.asarray(Wqkv), np.asarray(Wo))
    res = run_bass_kernel_spmd(nc, in_maps, core_ids=list(range(8)))
    return host_gather(res.results)


# revision 15
# speedup vs baseline: 1.1825x; 1.0720x over previous
"""TRN2 Bass/Tile kernel for nn_MultiHeadAttention_16647293239562.

kernel(x, Wqkv, Wo) -> [2, 2048, 1024] float32

Sharding: batch x head-group over 8 NeuronCores (core c: batch c//4, heads
4*(c%4)..4*(c%4)+3). Each core runs QKV projections (bf16 inputs, f32 PSUM
accumulation), RoPE (host-baked cos + pre-shifted sign-baked sin tables;
rotate-half via partition-shifted SBUF copies), full non-causal attention with
softmax denominators obtained from an extra ones-column in V (PV matmul row 64
= sum of exp), and a partial O-projection. Host sums 4 partials per batch.

Perf notes vs the original baseline:
- no DMA triggers on the scalar engine (it is saturated by exp; HWDGE
  triggers cost it ~670ns each) -> loads on sync + gpsimd, stores on gpsimd
- projections emitted n-chunk-outer so the in-order PE stream never blocks
  on an x chunk that is still loading while later-arriving work is ready
- softmax 1/denom: denominators DMA'd [1,1024]->[128,8] so one reciprocal
  runs with all DVE lanes active (a [1,512] reciprocal costs 3.3us)
- O-projection emitted with a lag of TWO chunks so its dependency on the
  serial normalize chain never stalls the in-order PE queue
- rotate-half shuffle DMAs in bf16 on the sync queue (half the bytes)
- V ones-column via on-device memset (saves a 1.3MB DMA); out_p in bf16
"""

import sys

sys.path.insert(0, "/opt/trn_rl_repo")

import concourse.bass as bass
import concourse.mybir as mybir
import concourse.tile as tile
from concourse import bacc

F32 = mybir.dt.float32
BF16 = mybir.dt.bfloat16
ts = bass.ts

L = 2048
D = 1024
DH = 64
NKT = D // 128   # 8 k-tiles
NLT = L // 128   # 16 l'-tiles
NQC = L // 512   # 4 q-chunks of 512
EXP = mybir.ActivationFunctionType.Exp


def build():
    nc = bacc.Bacc("TRN2", target_bir_lowering=False, debug=False)
    xTb = nc.dram_tensor("xTb", [D, L], BF16, kind="ExternalInput")
    wqkb = nc.dram_tensor("wqkb", [D, 512], BF16, kind="ExternalInput")
    wvb = nc.dram_tensor("wvb", [D, 256], BF16, kind="ExternalInput")
    wo = nc.dram_tensor("wo", [256, D], BF16, kind="ExternalInput")
    cosT = nc.dram_tensor("cosT", [128, L], F32, kind="ExternalInput")
    sinS = nc.dram_tensor("sinS", [128, L], F32, kind="ExternalInput")
    out_p = nc.dram_tensor("out_p", [L, D], BF16, kind="ExternalOutput")

    with tile.TileContext(nc) as tc:
        with tc.tile_pool(name="persist", bufs=1) as pers, \
             tc.tile_pool(name="mmpool", bufs=2, space="PSUM") as mmp, \
             tc.tile_pool(name="spool", bufs=2, space="PSUM") as spool, \
             tc.tile_pool(name="opool", bufs=1, space="PSUM") as opool, \
             tc.tile_pool(name="projsb", bufs=3) as psb, \
             tc.tile_pool(name="attn", bufs=18) as ap, \
             tc.tile_pool(name="nrm", bufs=3) as nrm, \
             tc.tile_pool(name="ost", bufs=4) as ost:
            # ---- persistent SBUF ----
            qkT = pers.tile([128, 2, 2, L], BF16, name="qkT")  # [128, q/k, ptile, L]
            vx = pers.tile([128, NLT, 260], BF16, name="vx")
            vx4 = vx.rearrange("p t (h c) -> p t h c", c=65)
            outT = pers.tile([128, 2, L], BF16, name="outT")
            cos_sb = pers.tile([128, L], F32, name="cos_sb")
            sin_sb = pers.tile([128, L], F32, name="sin_sb")
            xTb_sb = pers.tile([128, NKT, L], BF16, name="xTb_sb")
            wqkb_sb = pers.tile([128, NKT, 512], BF16, name="wqkb_sb")
            wvb_sb = pers.tile([128, NKT, 256], BF16, name="wvb_sb")
            wo_sb = pers.tile([128, 2, D], BF16, name="wo_sb")

            # ---- loads on sync HWDGE + gpsimd SWDGE queues (never scalar).
            # wqkb and the first x chunk interleaved so the first projection
            # matmul group can start as early as possible.
            xTbr = xTb.rearrange("(k p) l -> p k l", p=128)
            wqkbr = wqkb.rearrange("(k p) m -> p k m", p=128)
            for kt in range(NKT):
                eng = nc.sync if (kt % 2 == 0) else nc.gpsimd
                eng.dma_start(out=wqkb_sb[:, kt, :], in_=wqkbr[:, kt, :])
                eng2 = nc.gpsimd if (kt % 2 == 0) else nc.sync
                eng2.dma_start(out=xTb_sb[:, kt, ts(0, 512)], in_=xTbr[:, kt, ts(0, 512)])
            nc.gpsimd.dma_start(out=sin_sb, in_=sinS[:, :])
            for n in range(1, NQC):
                for kt in range(NKT):
                    eng = nc.sync if (kt % 2 == 0) else nc.gpsimd
                    eng.dma_start(out=xTb_sb[:, kt, ts(n, 512)], in_=xTbr[:, kt, ts(n, 512)])
                if n == 1:
                    nc.sync.dma_start(out=cos_sb, in_=cosT[:, :])
            nc.gpsimd.dma_start(out=wvb_sb, in_=wvb.rearrange("(k p) m -> p k m", p=128))
            nc.gpsimd.memset(vx4[:, :, :, 64:65], 1.0)  # ones column for denominators
            nc.gpsimd.dma_start(out=wo_sb, in_=wo.rearrange("(j p) d -> p j d", p=128))

            def qk_proj(m, n):
                # m: 0=q ptile0, 1=q ptile1, 2=k ptile0, 3=k ptile1
                qk, pt = (0, m) if m < 2 else (1, m - 2)
                ps = mmp.tile([128, 512], F32, tag="small", name="ps")
                for kt in range(NKT):
                    nc.tensor.matmul(ps, wqkb_sb[:, kt, ts(m, 128)],
                                     xTb_sb[:, kt, ts(n, 512)],
                                     start=(kt == 0), stop=(kt == NKT - 1))
                # u = ps * shift(sin); then shift(u) = shift(ps) * sin,
                # so no separate PSUM->SBUF copy is needed for the shift
                u = psb.tile([128, 512], BF16, tag="u", name="u")
                nc.vector.tensor_mul(u, ps, sin_sb[:, ts(n, 512)])
                rh = psb.tile([128, 512], BF16, tag="rh", name="rh")
                for blk in range(4):
                    src_p = blk * 32 + (32 if blk % 2 == 0 else -32)
                    nc.sync.dma_start(out=rh[blk * 32:(blk + 1) * 32, :],
                                      in_=u[src_p:src_p + 32, :])
                t0 = psb.tile([128, 512], BF16, tag="t0", name="t0")
                nc.vector.tensor_mul(t0, ps, cos_sb[:, ts(n, 512)])
                nc.vector.tensor_add(qkT[:, qk, pt, ts(n, 512)], t0, rh)

            def v_proj(lt):
                pv = mmp.tile([128, 256], F32, tag="small", padded_shape=[128, 512], name="pv")
                for kt in range(NKT):
                    nc.tensor.matmul(pv, xTb_sb[:, kt, ts(lt, 128)],
                                     wvb_sb[:, kt, :],
                                     start=(kt == 0), stop=(kt == NKT - 1))
                nc.vector.tensor_copy(vx4[:, lt, :, 0:64],
                                      pv.rearrange("p (h c) -> p h c", c=64))

            # Emission order: complete k (ptile0) and q chunk 0 FIRST so the
            # exp stream for attention chunk (pt0,qc0) starts as early as the
            # loads allow; V interleaves so PV(qc0) is never the laggard; the
            # remaining q chunks and ptile1 fill the exp-bound gaps after.
            qk_proj(2, 0)
            qk_proj(0, 0)
            qk_proj(2, 1)
            for lt in range(0, 4):
                v_proj(lt)
            qk_proj(2, 2)
            for lt in range(4, 8):
                v_proj(lt)
            qk_proj(2, 3)
            for lt in range(8, 16):
                v_proj(lt)
            for n in range(1, NQC):
                qk_proj(0, n)
            for n in range(NQC):
                qk_proj(3, n)
                qk_proj(1, n)

            def emit_outproj(qc2, subs=range(4)):
                # ---- output projection for one q-range (4 l-tiles) ----
                for sub in subs:
                    lt2 = qc2 * 4 + sub
                    for n in range(2):
                        po = mmp.tile([128, 512], F32, tag="small", name="po")
                        for j in range(2):
                            nc.tensor.matmul(po, outT[:, j, ts(lt2, 128)],
                                             wo_sb[:, j, ts(n, 512)],
                                             start=(j == 0), stop=(j == 1))
                        o_sb = ost.tile([128, 512], BF16, tag="o", name="o_sb")
                        nc.vector.tensor_copy(o_sb, po)
                        seng = nc.sync if (lt2 + n) % 2 == 0 else nc.gpsimd
                        seng.dma_start(out=out_p[ts(lt2, 128), ts(n, 512)], in_=o_sb)

            # ---- attention (pt-outer), high priority so the scheduler runs
            # scores/exp ASAP and packs V/ptile1-proj into the exp gaps
            for pt in range(2):
                for qc in range(NQC):
                  with tc.high_priority():
                    aABs = []
                    oA = opool.tile([65, 512], F32, tag="oA", name="oA")
                    oB = opool.tile([65, 512], F32, tag="oB", name="oB")
                    # phase 1: scores (both heads into one 2-bank tile) + one exp
                    for lt in range(NLT):
                        sAB = spool.tile([128, 1024], F32, tag="sAB", name="sAB")
                        nc.tensor.matmul(sAB[:, 0:512],
                                         qkT[0:64, 1, pt, ts(lt, 128)],
                                         qkT[0:64, 0, pt, ts(qc, 512)],
                                         start=True, stop=True)
                        nc.tensor.matmul(sAB[:, 512:1024],
                                         qkT[64:128, 1, pt, ts(lt, 128)],
                                         qkT[64:128, 0, pt, ts(qc, 512)],
                                         start=True, stop=True)
                        aAB = ap.tile([128, 1024], BF16, tag="aAB", name="aAB")
                        nc.scalar.activation(aAB, sAB, EXP, scale=0.125)
                        aABs.append(aAB)
                    # phase 2: PV accumulation (ones-column gives denominators)
                    h0, h1 = 2 * pt, 2 * pt + 1
                    for lt in range(NLT):
                        nc.tensor.matmul(oA, vx4[:, lt, h0, 0:65], aABs[lt][:, 0:512],
                                         start=(lt == 0), stop=(lt == NLT - 1))
                        nc.tensor.matmul(oB, vx4[:, lt, h1, 0:65], aABs[lt][:, 512:1024],
                                         start=(lt == 0), stop=(lt == NLT - 1))
                    # copy accumulators to SBUF so the PSUM slots free fast
                    ocd = nrm.tile([65, 1024], F32, tag="ocd", name="ocd")
                    nc.vector.tensor_copy(ocd[:, 0:512], oA)
                    nc.vector.tensor_copy(ocd[:, 512:1024], oB)
                    # normalize: out^T rows = o[0:64] * (1/o[64]); denominators
                    # go via [128,8] so the reciprocal uses all DVE lanes
                    dT = nrm.tile([128, 8], F32, tag="dT", name="dT")
                    nc.sync.dma_start(out=dT, in_=ocd[64:65, :])
                    rT = nrm.tile([128, 8], F32, tag="rT", name="rT")
                    nc.vector.reciprocal(rT, dT)
                    rAB = nrm.tile([1, 1024], F32, tag="rAB", name="rAB")
                    nc.sync.dma_start(out=rAB, in_=rT)
                    rbA = nrm.tile([64, 512], F32, tag="rbA", name="rbA")
                    rbB = nrm.tile([64, 512], F32, tag="rbB", name="rbB")
                    nc.gpsimd.partition_broadcast(rbA, rAB[:, 0:512])
                    nc.gpsimd.partition_broadcast(rbB, rAB[:, 512:1024])
                    last = (pt == 1 and qc == NQC - 1)
                    if last:
                        # lag-2 then lag-1: both sets of deps resolved during
                        # this chunk's scores phase
                        emit_outproj(qc - 2)
                        emit_outproj(qc - 1)
                        # drain the final chunk per 128-token subtile so the
                        # tail normalize/outproj/store chain pipelines
                        nB = nrm.tile([64, 512], BF16, tag="nB", name="nB")
                        for sub in range(4):
                            sl = slice(512 * qc + 128 * sub, 512 * qc + 128 * (sub + 1))
                            cl = slice(128 * sub, 128 * (sub + 1))
                            nc.vector.tensor_mul(outT[0:64, pt, sl],
                                                 ocd[0:64, cl], rbA[:, cl])
                            nc.vector.tensor_mul(nB[:, cl],
                                                 ocd[0:64, 512 + 128 * sub:512 + 128 * (sub + 1)],
                                                 rbB[:, cl])
                            nc.sync.dma_start(out=outT[64:128, pt, sl], in_=nB[:, cl])
                            emit_outproj(qc, [sub])
                    else:
                        nc.vector.tensor_mul(outT[0:64, pt, ts(qc, 512)], ocd[0:64, 0:512], rbA)
                        nB = nrm.tile([64, 512], BF16, tag="nB", name="nB")
                        nc.vector.tensor_mul(nB, ocd[0:64, 512:1024], rbB)
                        nc.sync.dma_start(out=outT[64:128, pt, ts(qc, 512)], in_=nB)

                    # lag-2 O-projection: its normalize deps resolved a full
                    # chunk ago, so it never stalls the in-order PE queue
                    if pt == 1 and 2 <= qc < NQC - 1:
                        emit_outproj(qc - 2)

    nc.finalize()
    return nc


def host_prepare(x, Wqkv, Wo):
    """Build the 8 per-core input maps from full inputs."""
    import numpy as np
    import ml_dtypes
    B = x.shape[0]
    inv_freq = 1.0 / (10000.0 ** (np.arange(0, DH, 2, dtype=np.float64) / DH))
    t = np.arange(L, dtype=np.float64)
    freqs = np.outer(t, inv_freq)               # [L, 32]
    emb = np.concatenate([freqs, freqs], -1)    # [L, 64]
    cosT_ = np.cos(emb).T                       # [64, L]
    sinT = np.sin(emb).T
    # pre-shifted sign-baked sin: g[dh] = (-sin if dh>=32 else +sin)[dh xor 32]
    sinSg = np.concatenate([sinT[32:64], -sinT[0:32]], axis=0)  # [64, L]
    cosT2 = np.ascontiguousarray(np.tile(cosT_, (2, 1)), dtype=np.float32)  # [128, L]
    sinS2 = np.ascontiguousarray(np.tile(sinSg, (2, 1)), dtype=np.float32)  # [128, L]

    xTb_h = [np.ascontiguousarray(x[b].T).astype(ml_dtypes.bfloat16) for b in range(B)]
    in_maps = []
    for c in range(8):
        b, hg = divmod(c, 4)
        hs = slice(4 * hg * DH, 4 * (hg + 1) * DH)  # 4 heads' col range (256 wide)
        wq = Wqkv[:, 0 * D:1 * D][:, hs]
        wk = Wqkv[:, 1 * D:2 * D][:, hs]
        wv_ = Wqkv[:, 2 * D:3 * D][:, hs]
        in_maps.append({
            "xTb": xTb_h[b],
            "wqkb": np.ascontiguousarray(
                np.concatenate([wq, wk], axis=1)).astype(ml_dtypes.bfloat16),
            "wvb": np.ascontiguousarray(wv_).astype(ml_dtypes.bfloat16),
            "wo": np.ascontiguousarray(Wo[hs, :]).astype(ml_dtypes.bfloat16),
            "cosT": cosT2,
            "sinS": sinS2,
        })
    return in_maps


def host_gather(results):
    """Sum partial outputs per batch -> [2, L, D] float32."""
    import numpy as np
    out = np.zeros((2, L, D), dtype=np.float32)
    for c, r in enumerate(results):
        out[c // 4] += r["out_p"].astype(np.float32)
    return out


_CACHED = {}


def kernel(x, Wqkv, Wo):
    import numpy as np
    from concourse.bass_utils import run_bass_kernel_spmd

    if "nc" not in _CACHED:
        _CACHED["nc"] = build()
    nc = _CACHED["nc"]
    in_maps = host_prepare(np.asarray(x), np.asarray(Wqkv), np.asarray(Wo))
    res = run_bass_kernel_spmd(nc, in_maps, core_ids=list(range(8)))
    return host_gather(res.results)
